# revision 12
# baseline (speedup 1.0000x reference)
"""Trainium2 Bass kernel for nn_HOOffset (SMPL-object offset PCA encode/decode +
Procrustes).

Math: the reference's two giant GEMMs against pca_components [128, 270336]
collapse algebraically. With off[b,(m,n,c)] = obj_rt[b,n,c] - smpl[b,m,c]:

  gamma[b,p] = sum_nc obj_rt[b,nc]*A[p,nc] - sum_mc smpl[b,mc]*B[p,mc] - mu[p]
     A[p,nc]  = sum_m pc[p,(m,n,c)]          (PE transpose-accumulate)
     B[p,mc]  = sum_n pc[p,(m,n,c)]          (DVE strided reduce)
     mu[p]    = sum_d mean[d]*pc[p,d]        (DVE fused mul-reduce)

  The decode+Procrustes sum S[b,n,c] = sum_m P[b,(m,n),c]
     = s_sum[b,c] + muA[n,c] + sum_p gamma[b,p]*A[p,nc]
  feeds svd_mat[b] = sum_n S[b,n,:]^T Qc[b,n,:] (host, with the 3x3 SVDs).

Sharding: column-shard pca_components over the 270336 dim = the m-anchor dim
(88 anchors/core); partial gammas all-reduced on device; per-core S partials
summed on host.

Each NeuronCore streams its 17.3MB pc shard from HBM exactly once; everything
else is tiny.  Expected bound: HBM DMA (~48us/core).
"""

import functools
from contextlib import ExitStack

import numpy as np

import concourse.bacc as bacc
import concourse.bass as bass
import concourse.mybir as mybir
import concourse.tile as tile
from concourse.bass_utils import run_bass_kernel_spmd
from concourse.masks import make_identity

FP = mybir.dt.float32

NCORES = 8
B = 64            # batch
P = 128           # pca components
M = 704           # smpl anchors
N = 128           # object anchors
CH = M // NCORES  # 88 anchors per core
NPC = N * 3       # 384  (n,c) block
MC = CH * 3       # 264  (m,c) block per core
D_CH = CH * NPC   # 33792 pc columns per core
K = 10            # betas
V = 10475         # smpl vertices
VP = 10496        # padded to 82*128
NVCH = VP // 128  # 82

PIECE_M = 8                    # m-anchors per streamed piece
PIECE_F = PIECE_M * NPC        # 3072 columns
NPIECE = CH // PIECE_M         # 11


def build_program():
    nc = bacc.Bacc("TRN2", target_bir_lowering=False, debug=False,
                   num_devices=NCORES)

    pc_d = nc.dram_tensor("pc", [P, D_CH], FP, kind="ExternalInput")
    mean_d = nc.dram_tensor("mean", [CH, NPC], FP, kind="ExternalInput")
    sdT_d = nc.dram_tensor("sdT", [K, MC], FP, kind="ExternalInput")
    vt_d = nc.dram_tensor("vt", [1, MC], FP, kind="ExternalInput")
    betasT_d = nc.dram_tensor("betasT", [K, B], FP, kind="ExternalInput")
    betas_d = nc.dram_tensor("betas", [B, K], FP, kind="ExternalInput")
    org_d = nc.dram_tensor("org", [B, NPC], FP, kind="ExternalInput")
    rot_d = nc.dram_tensor("rot", [B, 9], FP, kind="ExternalInput")
    trans_d = nc.dram_tensor("trans", [B, 3], FP, kind="ExternalInput")
    sdvt_d = nc.dram_tensor("sdvt", [P, NVCH * 33], FP, kind="ExternalInput")
    jreg_d = nc.dram_tensor("jreg", [P, NVCH], FP, kind="ExternalInput")

    gamma_o = nc.dram_tensor("gamma_out", [B, P], FP, kind="ExternalOutput")
    S_o = nc.dram_tensor("S_out", [B, NPC], FP, kind="ExternalOutput")

    with tile.TileContext(nc) as tc, ExitStack() as ctx:
        _body(ctx, tc, nc, pc_d, mean_d, sdT_d, vt_d, betasT_d, betas_d,
              org_d, rot_d, trans_d, sdvt_d, jreg_d, gamma_o, S_o)
    nc.compile()
    return nc


def _body(ctx, tc, nc, pc_d, mean_d, sdT_d, vt_d, betasT_d, betas_d,
          org_d, rot_d, trans_d, sdvt_d, jreg_d, gamma_o, S_o):
    consts = ctx.enter_context(tc.tile_pool(name="consts", bufs=1))
    small = ctx.enter_context(tc.tile_pool(name="small", bufs=1))
    pcpool = ctx.enter_context(tc.tile_pool(name="pcpool", bufs=3))
    mnpool = ctx.enter_context(tc.tile_pool(name="mnpool", bufs=2))
    mbpool = ctx.enter_context(tc.tile_pool(name="mbpool", bufs=2))
    scrpool = ctx.enter_context(tc.tile_pool(name="scrpool", bufs=2))
    psA = ctx.enter_context(tc.tile_pool(name="psA", bufs=3, space="PSUM"))
    psT = ctx.enter_context(tc.tile_pool(name="psT", bufs=2, space="PSUM"))
    psBig = ctx.enter_context(tc.tile_pool(name="psBig", bufs=2, space="PSUM"))
    dram = ctx.enter_context(tc.tile_pool(name="dram", bufs=1, space="DRAM"))

    # ---------------- constants / small loads ----------------
    from concourse import library_config
    nc.gpsimd.load_library(library_config.attn)

    ident = consts.tile([128, 128], FP)
    make_identity(nc, ident[:])

    betasT_s = consts.tile([K, B], FP)
    nc.sync.dma_start(betasT_s[:], betasT_d[:, :])
    betas_s = consts.tile([B, K], FP)
    nc.sync.dma_start(betas_s[:], betas_d[:, :])
    sdT_s = consts.tile([K, MC], FP)
    nc.sync.dma_start(sdT_s[:], sdT_d[:, :])
    vt_s = consts.tile([1, MC], FP)
    nc.sync.dma_start(vt_s[:], vt_d[:, :])
    org_s = consts.tile([B, NPC], FP)
    nc.sync.dma_start(org_s[:], org_d[:, :])
    rot_s = consts.tile([B, 9], FP)
    nc.sync.dma_start(rot_s[:], rot_d[:, :])
    trans_s = consts.tile([B, 3], FP)
    nc.sync.dma_start(trans_s[:], trans_d[:, :])
    sdvt_s = consts.tile([P, NVCH * 33], FP)
    nc.sync.dma_start(sdvt_s[:], sdvt_d[:, :])
    jreg_s = consts.tile([P, NVCH], FP)
    nc.sync.dma_start(jreg_s[:], jreg_d[:, :])
    mean_r = consts.tile([CH, NPC], FP)
    nc.sync.dma_start(mean_r[:], mean_d[:, :])

    ones_s = consts.tile([CH, 1], FP)
    nc.gpsimd.memset(ones_s[:], 1.0)

    # ---------------- phase 0: smpl anchors, obj_rt ----------------
    # W0/c0: [1,33] = sum_v jreg[v] * [shapedirs|v_template][v, 33]
    ps_w = psT.tile([1, 33], FP, tag="pst")
    for v in range(NVCH):
        nc.tensor.matmul(ps_w[:], jreg_s[:, v:v + 1], sdvt_s[:, v * 33:(v + 1) * 33],
                         start=(v == 0), stop=(v == NVCH - 1))
    w0c0 = small.tile([1, 33], FP)
    nc.scalar.copy(w0c0[:], ps_w[:])
    w0c0_b = small.tile([B, 33], FP)
    nc.gpsimd.partition_broadcast(w0c0_b[:], w0c0[:1, :])

    # J0[b,c] = c0[c] + sum_k betas[b,k] W0[c,k]
    J0 = small.tile([B, 3], FP)
    scrJ = small.tile([B, K], FP)
    for c in range(3):
        nc.vector.tensor_mul(scrJ[:], betas_s[:], w0c0_b[:, c * K:(c + 1) * K])
        nc.vector.tensor_reduce(out=J0[:, c:c + 1], in_=scrJ[:],
                                axis=mybir.AxisListType.X,
                                op=mybir.AluOpType.add)
    nc.vector.tensor_add(J0[:], J0[:], w0c0_b[:, 30:33])

    # smpl[b,(m,c)] = vt[(m,c)] + sum_k betas[b,k] sdT[k,(m,c)] - J0[b,c]
    ps_s = psBig.tile([B, MC], FP, tag="psbig")
    nc.tensor.matmul(ps_s[:], betasT_s[:], sdT_s[:], start=True, stop=True)
    vt_b = small.tile([B, MC], FP)
    nc.gpsimd.partition_broadcast(vt_b[:], vt_s[:1, :])
    smpl = small.tile([B, MC], FP)
    nc.vector.tensor_add(smpl[:], ps_s[:], vt_b[:])
    nc.vector.tensor_sub(
        smpl[:].rearrange("b (m c) -> b m c", c=3),
        smpl[:].rearrange("b (m c) -> b m c", c=3),
        J0[:, None, :].broadcast_to([B, CH, 3]))

    # s_sum[b,c] = sum_m smpl[b,m,c]
    s_sum = small.tile([B, 3], FP)
    nc.vector.tensor_reduce(
        out=s_sum[:], in_=smpl[:].rearrange("b (m c) -> b c m", c=3),
        axis=mybir.AxisListType.X, op=mybir.AluOpType.add)

    # smplT [mc, b] in chunks: [:,0:64]=mc 0..128, [:,64:128]=mc 128..256,
    # [:8,128:192]=mc 256..264
    smplT = small.tile([128, 192], FP)
    for j in range(2):
        pst = psT.tile([128, B], FP, tag="pst")
        nc.tensor.transpose(pst[:], smpl[:, j * 128:(j + 1) * 128], ident[:64, :64])
        nc.vector.tensor_copy(smplT[:, j * 64:(j + 1) * 64], pst[:])
    pst = psT.tile([128, B], FP, tag="pst")
    nc.tensor.transpose(pst[:8, :], smpl[:, 256:264], ident[:64, :64])
    nc.vector.tensor_copy(smplT[:8, 128:192], pst[:8, :])

    # obj_rt[b,(n,d)] = sum_c org[b,(n,c)] rot[b,(d,c)] + trans[b,d]
    objrt = small.tile([B, NPC], FP)
    org_v = org_s[:].rearrange("b (n c) -> b c n", c=3)
    objrt_v = objrt[:].rearrange("b (n c) -> b c n", c=3)
    acc = small.tile([B, N], FP)
    tmp = small.tile([B, N], FP)
    for d in range(3):
        nc.vector.tensor_scalar_mul(acc[:], org_v[:, 0], rot_s[:, 3 * d:3 * d + 1])
        for c in (1, 2):
            nc.vector.tensor_scalar_mul(tmp[:], org_v[:, c],
                                        rot_s[:, 3 * d + c:3 * d + c + 1])
            nc.vector.tensor_add(acc[:], acc[:], tmp[:])
        nc.vector.tensor_scalar_add(objrt_v[:, d], acc[:], trans_s[:, d:d + 1])

    # objrtT [nc, b]: chunks [:, j*64:(j+1)*64] = nc j*128..(j+1)*128
    objrtT = small.tile([128, 192], FP)
    for j in range(3):
        pst = psT.tile([128, B], FP, tag="pst")
        nc.tensor.transpose(pst[:], objrt[:, j * 128:(j + 1) * 128], ident[:64, :64])
        nc.vector.tensor_copy(objrtT[:, j * 64:(j + 1) * 64], pst[:])

    # muA[nc] = sum_m mean[(m,nc)]
    ps_mA = psT.tile([1, NPC], FP, tag="pst")
    nc.tensor.matmul(ps_mA[:], ones_s[:], mean_r[:], start=True, stop=True)
    muA = small.tile([1, NPC], FP)
    nc.scalar.copy(muA[:], ps_mA[:])
    muA_b = small.tile([B, NPC], FP)
    nc.gpsimd.partition_broadcast(muA_b[:], muA[:1, :])

    # ---------------- phase 1: stream pc shard ----------------
    # A^T accumulated by PE transposes; B by DVE strided reduce; mu by DVE
    # fused mul-reduce against partition-broadcast mean.
    psA_t = [psA.tile([128, 128], FP, tag="psA", name=f"psA{j}") for j in range(3)]
    B_s = consts.tile([P, MC], FP)
    mu_parts = consts.tile([P, NPIECE], FP)
    mu_acc = consts.tile([P, 1], FP)

    for t in range(NPIECE):
        pc_t = pcpool.tile([P, PIECE_F], FP)
        nc.sync.dma_start(pc_t[:], pc_d[:, t * PIECE_F:(t + 1) * PIECE_F])

        mn_t = mnpool.tile([1, PIECE_F], FP)
        nc.sync.dma_start(mn_t[:], mean_d[t * PIECE_M:(t + 1) * PIECE_M, :])
        mb_t = mbpool.tile([P, PIECE_F], FP)
        nc.gpsimd.partition_broadcast(mb_t[:], mn_t[:1, :])

        scr_t = scrpool.tile([P, PIECE_F], FP)
        nc.vector.tensor_mul(scr_t[:], pc_t[:], mb_t[:])
        nc.vector.tensor_reduce(out=mu_parts[:, t:t + 1], in_=scr_t[:],
                                axis=mybir.AxisListType.X,
                                op=mybir.AluOpType.add)

        nc.vector.tensor_reduce(
            out=B_s[:, t * PIECE_M * 3:(t + 1) * PIECE_M * 3]
                .rearrange("p (m c) -> p m c", c=3),
            in_=pc_t[:].rearrange("p (m n c) -> p m c n", m=PIECE_M, n=N, c=3),
            axis=mybir.AxisListType.X, op=mybir.AluOpType.add)

        for mm in range(PIECE_M):
            base = mm * NPC
            for j in range(3):
                nc.tensor.matmul(
                    psA_t[j][:], pc_t[:, base + j * 128:base + (j + 1) * 128],
                    ident[:], is_transpose=True,
                    start=(t == 0 and mm == 0), stop=(t == NPIECE - 1 and mm == PIECE_M - 1))

    # ---------------- phase 2: gamma, all-reduce, S ----------------
    AT_s = consts.tile([P, NPC], FP)      # A^T: [:, j*128:...] = nc block j ([nc128, p128])
    for j in range(3):
        nc.vector.tensor_copy(AT_s[:, j * 128:(j + 1) * 128], psA_t[j][:])
    A_s = consts.tile([P, NPC], FP)       # A natural [p, nc]
    for j in range(3):
        pst = psT.tile([128, 128], FP, tag="pst")
        nc.tensor.transpose(pst[:], AT_s[:, j * 128:(j + 1) * 128], ident[:])
        nc.vector.tensor_copy(A_s[:, j * 128:(j + 1) * 128], pst[:])

    # -B^T chunks: [:,0:128],[ :,128:256] = mc blocks 0,1 ([mc128, p128]);
    # [:8, 256:384] = mc 256..264
    BnT = small.tile([128, NPC], FP)
    for j in range(2):
        pst = psT.tile([128, 128], FP, tag="pst")
        nc.tensor.transpose(pst[:], B_s[:, j * 128:(j + 1) * 128], ident[:])
        nc.scalar.mul(BnT[:, j * 128:(j + 1) * 128], pst[:], -1.0)
    pst = psT.tile([128, 128], FP, tag="pst")
    nc.tensor.transpose(pst[:8, :], B_s[:, 256:264], ident[:])
    nc.scalar.mul(BnT[:8, 256:384], pst[:8, :], -1.0)

    # gamma_partial = objrt @ A^T - smpl @ B^T - mu
    ps_g = psBig.tile([B, P], FP, tag="psbig")
    for j in range(3):
        nc.tensor.matmul(ps_g[:], objrtT[:, j * 64:(j + 1) * 64],
                         AT_s[:, j * 128:(j + 1) * 128],
                         start=(j == 0), stop=False)
    for j in range(2):
        nc.tensor.matmul(ps_g[:], smplT[:, j * 64:(j + 1) * 64],
                         BnT[:, j * 128:(j + 1) * 128], start=False, stop=False)
    nc.tensor.matmul(ps_g[:], smplT[:8, 128:192], BnT[:8, 256:384],
                     start=False, stop=True)

    nc.vector.tensor_reduce(out=mu_acc[:, 0:1], in_=mu_parts[:],
                            axis=mybir.AxisListType.X, op=mybir.AluOpType.add)
    mu_row = small.tile([1, P], FP)
    nc.gpsimd.dma_start(out=mu_row[:1, :], in_=mu_acc[:, 0:1])
    mu_b = small.tile([B, P], FP)
    nc.gpsimd.partition_broadcast(mu_b[:], mu_row[:1, :])
    gpart = small.tile([B, P], FP)
    nc.vector.tensor_sub(gpart[:], ps_g[:], mu_b[:])

    g_in = dram.tile([B, P], FP)
    g_out = dram.tile([B, P], FP)
    nc.gpsimd.dma_start(g_in[:], gpart[:])
    nc.gpsimd.collective_compute(
        "AllReduce", mybir.AluOpType.add,
        replica_groups=[list(range(NCORES))],
        ins=[g_in[:].opt()], outs=[g_out[:].opt()])
    gfull = small.tile([B, P], FP)
    nc.gpsimd.dma_start(gfull[:], g_out[:])
    nc.gpsimd.dma_start(gamma_o[:, :], g_out[:])

    # S[b,nc] = s_sum[b,c] + muA[nc] + sum_p gamma[b,p] A[p,nc]
    gT = small.tile([P, B], FP)
    pst = psT.tile([P, B], FP, tag="pst")
    nc.tensor.transpose(pst[:], gfull[:], ident[:64, :64])
    nc.vector.tensor_copy(gT[:], pst[:])
    ps_S = psBig.tile([B, NPC], FP, tag="psbig")
    nc.tensor.matmul(ps_S[:], gT[:], A_s[:], start=True, stop=True)
    S_s = small.tile([B, NPC], FP)
    nc.vector.tensor_add(S_s[:], ps_S[:], muA_b[:])
    nc.vector.tensor_add(
        S_s[:].rearrange("b (n c) -> b n c", c=3),
        S_s[:].rearrange("b (n c) -> b n c", c=3),
        s_sum[:, None, :].broadcast_to([B, N, 3]))
    nc.sync.dma_start(S_o[:, :], S_s[:])


@functools.lru_cache(maxsize=1)
def _get_nc():
    return build_program()


def make_in_maps(inputs):
    betas = np.ascontiguousarray(inputs["smpl_betas"], np.float32)
    rot = np.ascontiguousarray(inputs["object_rel_rotmat"], np.float32)
    trans = np.ascontiguousarray(inputs["object_rel_trans"], np.float32)
    org = np.ascontiguousarray(inputs["object_anchors_org"], np.float32)
    vt = np.ascontiguousarray(inputs["v_template"], np.float32)
    sd = np.ascontiguousarray(inputs["shapedirs"], np.float32)
    Jr = np.ascontiguousarray(inputs["J_regressor"], np.float32)
    mean = np.ascontiguousarray(inputs["pca_mean"], np.float32)
    pc = np.ascontiguousarray(inputs["pca_components"], np.float32)
    anch = np.asarray(inputs["anchor_indices"])

    jreg_pad = np.zeros(VP, np.float32)
    jreg_pad[:V] = Jr[0]
    jreg = np.ascontiguousarray(jreg_pad.reshape(NVCH, 128).T)
    sdvt_pad = np.zeros((VP, 33), np.float32)
    sdvt_pad[:V, :30] = sd.reshape(V, 30)
    sdvt_pad[:V, 30:] = vt
    sdvt = np.ascontiguousarray(
        sdvt_pad.reshape(NVCH, 128, 33).transpose(1, 0, 2).reshape(128, NVCH * 33))

    rep = dict(
        betasT=np.ascontiguousarray(betas.T),
        betas=betas,
        org=np.ascontiguousarray(org.reshape(B, NPC)),
        rot=np.ascontiguousarray(rot.reshape(B, 9)),
        trans=trans,
        sdvt=sdvt,
        jreg=jreg,
    )
    in_maps = []
    for i in range(NCORES):
        idx = anch[CH * i:CH * (i + 1)]
        m = dict(rep)
        m["pc"] = np.ascontiguousarray(pc[:, D_CH * i:D_CH * (i + 1)])
        m["mean"] = np.ascontiguousarray(
            mean[D_CH * i:D_CH * (i + 1)].reshape(CH, NPC))
        m["sdT"] = np.ascontiguousarray(
            sd[idx].transpose(2, 0, 1).reshape(K, MC))
        m["vt"] = np.ascontiguousarray(vt[idx].reshape(1, MC))
        in_maps.append(m)
    return in_maps


def postprocess(inputs, gamma, S_parts):
    S = np.zeros((B, NPC), np.float64)
    for s in S_parts:
        S += s.astype(np.float64)
    S = S.reshape(B, N, 3)
    org = np.asarray(inputs["object_anchors_org"], np.float64)
    meanQ = org.mean(axis=1)
    Qc = org - meanQ[:, None, :]
    svd_mat = np.einsum("bnc,bnd->bcd", S, Qc)
    u, s, vh = np.linalg.svd(svd_mat)
    d = np.linalg.det(u @ vh)
    dvec = np.stack([np.ones_like(d), np.ones_like(d), d], axis=-1)
    R = (u * dvec[:, None, :]) @ vh
    T = S.sum(axis=1) / (M * N) - np.einsum("bij,bj->bi", R, meanQ)
    return (np.asarray(gamma, np.float32), R.astype(np.float32),
            T.astype(np.float32))


def kernel(**inputs):
    in_maps = make_in_maps(inputs)
    nc = _get_nc()
    res = run_bass_kernel_spmd(nc, in_maps, list(range(NCORES)))
    results = res.results
    gamma = results[0]["gamma_out"]
    S_parts = [results[i]["S_out"] for i in range(NCORES)]
    return postprocess(inputs, gamma, S_parts)
